# revision 53
# baseline (speedup 1.0000x reference)
"""Trainium2 Bass kernel for a dense transformer block (B=2, T=2048, C=1024, H=16).

Sharding (pipelined tensor-parallel attention + chunked ReduceScatter):
  core c -> batch b = c//4, head-group g = c%4 (heads 4g..4g+3).
  After the chunked proj ReduceScatter, core g owns the scattered token
  rows {512*w + 128*g : w in 0..3} of its batch for the MLP phase.

Pipeline (w = 512-token window / query chunk):
  LN1(window w) -> 16x128 xbar DMA transpose -> qkv(window w) ->
  attention for queries w over keys 0..w (block-causal exact,
  live-range trimmed diagonal tiles, single [128,128] staircase mask,
  softmax denominator via a ones-column in v, scores software-pipelined
  one pair ahead of the AV matmuls) -> normalize -> proj partial
  (token-major) -> per-window ReduceScatter(add) over the 4-core batch
  group.  The four chunk collectives run on the collective queue
  underneath the remaining attention compute; only the last chunk's RS
  is exposed.  Residual + LN2 per 128-row chunk (rs reads ride the Pool
  SWDGE queue so their waits never block the SP/ACT queues); fc1 runs
  in two row-groups (chunks 0-2 can start while the last RS is still in
  flight, chunk 3 after); fc2 full width; fc2 outputs are
  xbar-transposed back to token-major and added to the residual.

All transposes are xbar DMA transposes (bf16).  LN rsqrt is ACT Sqrt +
DVE reciprocal up front and a DVE-only Babylonian iteration inside the
attention stream (keeps the exp activation table resident).  LayerNorm
affines and b_proj/b_v are folded into the adjacent matmul weights /
the residual input on the host (exact).  All matmuls are bf16 with
fp32 PSUM accumulation.  PSUM budget in the attention loop:
qkv(2) + scores(4) + av/proj(2) = 8 banks.
"""

import contextlib
from contextlib import ExitStack

import ml_dtypes
import numpy as np

import concourse.bass as bass
import concourse.tile as tile
import concourse.bacc as bacc
import concourse.mybir as mybir
from concourse.bass_utils import run_bass_kernel_spmd

F32 = mybir.dt.float32
BF16 = mybir.dt.bfloat16
ALU = mybir.AluOpType
ACTF = mybir.ActivationFunctionType

B, T, C = 2, 2048, 1024
H, DH = 16, 64
FF = 4096
EPS = 1e-5
NCORES = 8
ROWS = 512            # token rows owned per core (MLP phase)
HG = 4                # heads per core
HGF = HG * DH         # 256 head-group features
NTT = T // 128        # 16 token tiles
NCP = C // 128        # 8 feature partition-tiles of C
NFP = FF // 128       # 32 feature partition-tiles of FF
NQC = T // 512        # 4 query chunks / token windows
VSTRIDE = DH + 1      # v stored with a ones column per head


def r(ap, pat, **kw):
    return ap.rearrange(pat, **kw)


def build_program():
    nc = bacc.Bacc("TRN2", target_bir_lowering=False, debug=False,
                   num_devices=NCORES)

    def din(name, shape, dtype=F32):
        return nc.dram_tensor(name, list(shape), dtype, kind="ExternalInput")

    xb = din("xb", (T, C), BF16)              # LN1 input (bf16 copy of x)
    xo_d = din("xo", (ROWS, C), BF16)         # own rows of x (residual)
    cmask = din("cmask", (128, 128), BF16)    # staircase: m[p,j] = j >= p
    w_qkv = din("w_qkv", (128, NCP, 3 * HGF), BF16)   # [p][kt][q|k|v]
    w_pr = din("w_pr", (HGF, C), BF16)        # proj rows for this head group
    w_fc = din("w_fc", (128, NFP * NCP * 128), BF16)  # [p][m][kt][c]
    w_fc2 = din("w_fc2", (128, NCP * NFP * 128), BF16)  # [p][m2][k2][c]
    b_qk_col = din("b_qk_col", (128, 4))      # q0 q1 k0 k1 bias columns
    b_v_bc = din("b_v_bc", (128, HGF), BF16)
    b_fc_col = din("b_fc_col", (128, NFP))
    b_fc2_col = din("b_fc2_col", (128, NCP))
    out = nc.dram_tensor("out", [ROWS, C], F32, kind="ExternalOutput")

    with tile.TileContext(nc) as tc, ExitStack() as ctx:
        # ---- constants ----
        cpool = ctx.enter_context(tc.tile_pool(name="const", bufs=1))
        bqk = cpool.tile([128, 4], F32, tag="bqk")
        bvbc = cpool.tile([128, HGF], BF16, tag="bvbc")
        bfc = cpool.tile([128, NFP], F32, tag="bfc")
        bfc2 = cpool.tile([128, NCP], F32, tag="bfc2")
        mtile = cpool.tile([128, 128], BF16, tag="mtile")
        epsc = cpool.tile([128, 1], F32, tag="epsc")
        nc.gpsimd.memset(epsc[:], EPS)

        def load_late_consts():
            nc.sync.dma_start(bfc[:], b_fc_col.ap())
            nc.sync.dma_start(bfc2[:], b_fc2_col.ap())

        def rsqrt_eps(spool, var_ap, tag):
            """rc = 1/sqrt(var+eps) via ACT Sqrt + DVE reciprocal."""
            sd = spool.tile([128, 1], F32, tag=f"sd{tag}")
            nc.scalar.activation(sd[:], var_ap, ACTF.Sqrt, bias=epsc[:],
                                 scale=1.0)
            rc = spool.tile([128, 1], F32, tag=f"rc{tag}")
            nc.vector.reciprocal(rc[:], sd[:])
            return rc

        def rsqrt_eps_dve(spool, var_ap, tag):
            """rc = 1/sqrt(var+eps) entirely on DVE (Babylonian iteration;
            var is O(1) here so 3 iterations from s0=(u+1)/2 are exact to
            ~1e-6).  Keeps ACT free of non-exp table sets mid-attention."""
            u = spool.tile([128, 1], F32, tag=f"u{tag}")
            nc.vector.tensor_scalar(u[:], var_ap, EPS, None, op0=ALU.add)
            s = spool.tile([128, 1], F32, tag=f"s{tag}")
            nc.vector.tensor_scalar(s[:], u[:], 1.0, 0.5, op0=ALU.add,
                                    op1=ALU.mult)
            t = spool.tile([128, 1], F32, tag=f"t{tag}")
            m = spool.tile([128, 1], F32, tag=f"m{tag}")
            for _ in range(3):
                nc.vector.reciprocal(t[:], s[:])
                nc.vector.tensor_tensor(m[:], u[:], t[:], op=ALU.mult)
                nc.vector.tensor_scalar(s[:], s[:], m[:], 0.5, op0=ALU.add,
                                        op1=ALU.mult)
            rc = spool.tile([128, 1], F32, tag=f"rc{tag}")
            nc.vector.reciprocal(rc[:], s[:])
            return rc

        def ln_stats(spool, xt, tag):
            """Returns ag [128,2] = (mean, var) over the 1024 free elems."""
            st = spool.tile([128, 12], F32, tag=f"st{tag}")
            nc.vector.bn_stats(st[:, 0:6], xt[:, 0:512])
            nc.vector.bn_stats(st[:, 6:12], xt[:, 512:1024])
            ag = spool.tile([128, 2], F32, tag=f"ag{tag}")
            nc.vector.bn_aggr(ag[:], r(st, "p (c s) -> p c s", s=6))
            return ag

        # persistent SBUF (lives into the MLP phase)
        app = ctx.enter_context(tc.tile_pool(name="attp", bufs=1))
        x2 = [app.tile([128, C], F32, tag=f"x2{i}", name=f"x2{i}")
              for i in range(NQC)]
        # feature-major LN2 output: chunks 0-2 in one tile (fc1 row-group
        # a reads them as one N=384 moving operand), chunk 3 separate
        xn2Ta = app.tile([128, 3, NCP, 128], BF16, tag="xn2Ta", name="xn2Ta")
        xn2Tb = app.tile([128, NCP, 128], BF16, tag="xn2Tb", name="xn2Tb")

        wfcp = ctx.enter_context(tc.tile_pool(name="wfc", bufs=1))
        wfc = wfcp.tile([128, NFP * NCP * 128], BF16, tag="wfc", name="wfc")

        # DRAM bounce buffers for the chunked collective
        drpool = ctx.enter_context(tc.tile_pool(name="dram", bufs=1,
                                                space="DRAM"))
        pp_d = [drpool.tile([ROWS, C], BF16, tag=f"pp{w}", name=f"pp{w}")
                for w in range(NQC)]
        rs_d = [drpool.tile([128, C], BF16, tag=f"rs{w}", name=f"rs{w}")
                for w in range(NQC)]

        with ExitStack() as qctx:
            atb = qctx.enter_context(tc.tile_pool(name="attb", bufs=1))
            kTb = [atb.tile([128, T], BF16, tag=f"kT{i}", name=f"kT{i}")
                   for i in range(2)]
            vb = [atb.tile([128, HG * VSTRIDE], BF16, tag=f"v{i}",
                           name=f"v{i}") for i in range(NTT)]
            qyp = qctx.enter_context(tc.tile_pool(name="qy", bufs=2))
            yyp = qctx.enter_context(tc.tile_pool(name="yy", bufs=1))
            cur = {}

            lnp = qctx.enter_context(tc.tile_pool(name="ln1", bufs=8))
            lnxn = qctx.enter_context(tc.tile_pool(name="ln1x", bufs=6))
            lns = qctx.enter_context(tc.tile_pool(name="ln1s", bufs=8))
            xnTp = qctx.enter_context(tc.tile_pool(name="xnT", bufs=2))
            xnTs = {}
            wqp = qctx.enter_context(tc.tile_pool(name="wq", bufs=1))
            wq = wqp.tile([128, NCP, 3 * HGF], BF16, tag="wq", name="wq")
            wpp = qctx.enter_context(tc.tile_pool(name="wp", bufs=1))
            wp = [wpp.tile([128, C], BF16, tag=f"wp{i}", name=f"wp{i}")
                  for i in range(2)]

            qkps = qctx.enter_context(tc.tile_pool(name="qkps", bufs=2,
                                                   space="PSUM"))
            scps = qctx.enter_context(tc.tile_pool(name="scps", bufs=2,
                                                   space="PSUM"))
            avps = qctx.enter_context(tc.tile_pool(name="avps", bufs=1,
                                                   space="PSUM"))
            atp = qctx.enter_context(tc.tile_pool(name="att", bufs=2))
            smp = qctx.enter_context(tc.tile_pool(name="attsm", bufs=1))
            pep = qctx.enter_context(tc.tile_pool(name="pe", bufs=1))
            esp = qctx.enter_context(tc.tile_pool(name="ep", bufs=3))
            exp2 = qctx.enter_context(tc.tile_pool(name="ep2", bufs=2))
            ess = qctx.enter_context(tc.tile_pool(name="eps", bufs=3))

            xns = {}

            def ln1_compute(w):
                """x loads (SP) + LN1 stats/apply (DVE/ACT) for window w."""
                for tl in range(4):
                    tt = 4 * w + tl
                    xt = lnp.tile([128, C], BF16, tag="x")
                    nc.sync.dma_start(xt[:],
                                      xb.ap()[tt * 128:(tt + 1) * 128, :])
                    if tt == 3:
                        q4 = NCP * 3 * HGF // 4
                        wqf = r(wq[:], "p a b -> p (a b)")
                        wqs = r(w_qkv.ap(), "p a b -> p (a b)")
                        for ci in range(4):
                            nc.sync.dma_start(wqf[:, ci * q4:(ci + 1) * q4],
                                              wqs[:, ci * q4:(ci + 1) * q4])
                        nc.sync.dma_start(bqk[:], b_qk_col.ap())
                        nc.sync.dma_start(bvbc[:], b_v_bc.ap())
                        nc.sync.dma_start(mtile[:], cmask.ap())
                        for i in range(2):
                            nc.sync.dma_start(
                                wp[i][:], w_pr.ap()[i * 128:(i + 1) * 128])
                    ag = ln_stats(lns, xt[:], "a")
                    rc = rsqrt_eps(lns, ag[:, 1:2], "a")
                    nmr = lns.tile([128, 1], F32, tag="nmr")
                    nc.vector.tensor_scalar(nmr[:], ag[:, 0:1], rc[:], -1.0,
                                            op0=ALU.mult, op1=ALU.mult)
                    xn = lnxn.tile([128, C], BF16, tag="xn")
                    nc.scalar.activation(xn[:], xt[:], ACTF.Identity,
                                         bias=nmr[:], scale=rc[:])
                    xns[tt] = xn

            def ln1_transpose(w):
                xnT = xnTp.tile([128, 4, NCP, 128], BF16, tag="xnT",
                                name=f"xnT{w}")
                xnTs[w] = xnT
                for tl in range(4):
                    nc.sync.dma_start(xnT[:, tl, :, :],
                                      xns.pop(4 * w + tl)[:],
                                      transpose=True)

            def qkv_window(w):
                """q,k (feature-major) + v (token-major) for window w."""
                xnT = xnTs.pop(w)
                cur["qT"] = [qyp.tile([128, 512], BF16, tag=f"qTw{i}",
                                      name=f"qT_{w}_{i}") for i in range(2)]
                for m in range(4):
                    ps = qkps.tile([128, 512], F32, tag="qk")
                    for kt in range(NCP):
                        nc.tensor.matmul(
                            ps[:],
                            wq[:, kt, m * 128:(m + 1) * 128],
                            xnT[:, :, kt, :],
                            start=(kt == 0), stop=(kt == NCP - 1))
                    if m < 2:
                        dst = cur["qT"][m][:, :]
                    else:
                        dst = kTb[m % 2][:, w * 512:(w + 1) * 512]
                    nc.vector.tensor_scalar(
                        dst, ps[:],
                        bqk[:, m:m + 1], None, op0=ALU.add)
                for tl in range(4):
                    tt = 4 * w + tl
                    ps = qkps.tile([128, 512], F32, tag="qk")
                    for kt in range(NCP):
                        nc.tensor.matmul(
                            ps[:, 0:HGF],
                            xnT[:, tl, kt, :],
                            wq[:, kt, 2 * HGF:3 * HGF],
                            start=(kt == 0), stop=(kt == NCP - 1))
                    dst = r(vb[tt], "p (h m) -> p h m",
                            m=VSTRIDE)[:, :, 0:DH]
                    nc.vector.tensor_tensor(
                        dst, r(ps[:, 0:HGF], "p (h m) -> p h m", m=DH),
                        r(bvbc[:], "p (h m) -> p h m", m=DH), op=ALU.add)

            def attention_qc(qc, inject={}):
                """Queries [512qc, 512qc+512) for the 4 heads; exact
                block-causal with live-range trimmed diagonal tiles."""
                nkt = 4 * (qc + 1)
                qT = cur["qT"]
                cur["yT"] = [yyp.tile([128, 512], BF16, tag=f"yTw{i}",
                                      name=f"yT_{qc}_{i}") for i in range(2)]

                pairs = [(pt, kp, sub) for pt in range(2)
                         for kp in range(nkt // 2) for sub in range(2)]
                avs = {}
                sc_et = {}

                def segs_of(kp):
                    out = []
                    for j in range(2):
                        kt = 2 * kp + j
                        band = kt - 4 * qc
                        lo = 128 * band if band > 0 else 0
                        out.append((kt, band, lo,
                                    slice(j * 512 + lo, (j + 1) * 512)))
                    return out

                def scores(pair):
                    pt, kp, sub = pair
                    hb = 64 * sub
                    sc = scps.tile([128, 1024], F32, tag="sc")
                    et = atp.tile([128, 1024], BF16, tag="e")
                    sc_et[pair] = (sc, et)
                    for kt, band, lo, seg in segs_of(kp):
                        nc.tensor.matmul(
                            sc[:, seg],
                            kTb[pt][hb:hb + 64, kt * 128:(kt + 1) * 128],
                            qT[pt][hb:hb + 64, lo:512],
                            start=True, stop=True)

                def expmask(pair):
                    pt, kp, sub = pair
                    sc, et = sc_et[pair]
                    sg = segs_of(kp)
                    if sg[0][1] < 0 and sg[1][1] < 0:
                        nc.scalar.activation(et[:, 0:1024], sc[:, 0:1024],
                                             ACTF.Exp, scale=0.125)
                    else:
                        for kt, band, lo, seg in sg:
                            nc.scalar.activation(et[:, seg], sc[:, seg],
                                                 ACTF.Exp, scale=0.125)
                    for kt, band, lo, seg in sg:
                        if band >= 0:
                            j = kt - 2 * kp
                            dsl = slice(j * 512 + lo, j * 512 + lo + 128)
                            nc.vector.tensor_tensor(
                                et[:, dsl], et[:, dsl], mtile[:],
                                op=ALU.mult)

                def av(pair):
                    pt, kp, sub = pair
                    h = 2 * pt + sub
                    sc, et = sc_et.pop(pair)
                    if (pt, sub) not in avs:
                        avs[(pt, sub)] = avps.tile(
                            [128, 512], F32, tag=f"av{sub}",
                            name=f"av_{qc}_{pt}_{sub}")
                    for kt, band, lo, seg in segs_of(kp):
                        nc.tensor.matmul(
                            avs[(pt, sub)][0:VSTRIDE, lo:512],
                            vb[kt][:, h * VSTRIDE:(h + 1) * VSTRIDE],
                            et[:, seg],
                            start=(kt == 0), stop=(kt == nkt - 1),
                            skip_group_check=True)

                def normalize(pt):
                    for sub in range(2):
                        hb = 64 * sub
                        a = avs.pop((pt, sub))
                        rr = smp.tile([1, 512], F32, tag="rr")
                        nc.vector.reciprocal(rr[:], a[DH:DH + 1, :])
                        bc = smp.tile([64, 512], F32, tag="bc")
                        nc.gpsimd.partition_broadcast(bc[:], rr[:])
                        nc.vector.tensor_tensor(
                            cur["yT"][pt][hb:hb + 64, :],
                            a[0:DH, :], bc[:], op=ALU.mult)

                scores(pairs[0])
                for i, pair in enumerate(pairs):
                    fn = inject.pop(i, None)
                    if fn is not None:
                        fn()
                    expmask(pair)
                    if i + 1 < len(pairs):
                        scores(pairs[i + 1])
                    av(pair)
                    if pair[0] == 0 and i + 1 < len(pairs) \
                            and pairs[i + 1][0] == 1:
                        normalize(0)
                normalize(1)

            def proj_chunk(w):
                """Token-major proj partial for window w + chunk RS."""
                pe = pep.tile([128, 4, C], BF16, tag="pe")
                yT = cur["yT"]
                for tl in range(4):
                    for cc in range(2):
                        ps = avps.tile([128, 512], F32,
                                       tag=f"av{(2 * tl + cc) % 2}",
                                       name=f"pj_{w}_{tl}_{cc}")
                        for i in range(2):
                            nc.tensor.matmul(
                                ps[:],
                                yT[i][:, tl * 128:(tl + 1) * 128],
                                wp[i][:, cc * 512:(cc + 1) * 512],
                                start=(i == 0), stop=(i == 1))
                        if cc == 0:
                            nc.vector.tensor_copy(
                                pe[:, tl, cc * 512:(cc + 1) * 512], ps[:])
                        else:
                            nc.scalar.copy(
                                pe[:, tl, cc * 512:(cc + 1) * 512], ps[:])

                # pp write from the Pool SWDGE queue: its wait never blocks
                # the SP stream or the ACT exp stream
                nc.gpsimd.dma_start(
                    r(pp_d[w][:, :], "(tl p) c -> p tl c", p=128), pe[:])
                nc.gpsimd.collective_compute(
                    "ReduceScatter", ALU.add,
                    replica_groups=[[0, 1, 2, 3], [4, 5, 6, 7]],
                    ins=[pp_d[w].opt()], outs=[rs_d[w].opt()])

            xn2s = {}

            rs_sbs = {}

            def epilogue_read(w):
                """rs read (Pool swdge: reached only after the next chunk's
                collective trigger, so its wait never blocks) + xo load."""
                rs_sb = esp.tile([128, C], BF16, tag="rs")
                nc.gpsimd.dma_start(rs_sb[:], rs_d[w][:, :])
                xot = esp.tile([128, C], BF16, tag="xot")
                nc.sync.dma_start(xot[:],
                                  xo_d.ap()[w * 128:(w + 1) * 128, :])
                rs_sbs[w] = (rs_sb, xot)

            def epilogue_compute(w, defer=0):
                """residual + LN2 (DVE only).  ``defer`` pushes the ops'
                scheduler priority later so their RS-gated waits never clog
                the in-order DVE queue ahead of attention work."""
                with tc.high_priority(offset=-defer) if defer else \
                        contextlib.nullcontext():
                    _epilogue_compute(w)

            def _epilogue_compute(w):
                rs_sb, xot = rs_sbs.pop(w)
                nc.vector.tensor_tensor(x2[w][:], rs_sb[:], xot[:],
                                        op=ALU.add)
                ag = ln_stats(ess, x2[w][:], "e")
                rc = rsqrt_eps_dve(ess, ag[:, 1:2], "e")
                xn2 = exp2.tile([128, C], BF16, tag="xn2")
                nc.vector.tensor_scalar(xn2[:], x2[w][:], ag[:, 0:1], rc[:],
                                        op0=ALU.subtract, op1=ALU.mult)
                xn2s[w] = xn2

            def epilogue_transpose(w):
                dst = xn2Ta[:, w, :, :] if w < 3 else xn2Tb[:, :, :]
                nc.sync.dma_start(dst, xn2s.pop(w)[:], transpose=True)

            # ---------------- pipelined main loop ----------------
            for tt in range(NTT):
                nc.gpsimd.memset(
                    r(vb[tt], "p (h m) -> p h m",
                      m=VSTRIDE)[:, :, DH:DH + 1], 1.0)
            ln1_compute(0)
            ln1_compute(1)
            for w in range(NQC):
                if 1 <= w < NQC - 1:
                    ln1_compute(w + 1)
                ln1_transpose(w)
                qkv_window(w)
                if w == NQC - 1:
                    nch = 8
                    csz = NFP * NCP * 128 // nch
                    for ci in range(nch):
                        nc.sync.dma_start(wfc[:, ci * csz:(ci + 1) * csz],
                                          w_fc.ap()[:, ci * csz:(ci + 1) * csz])
                    load_late_consts()
                if w >= 1:
                    epilogue_read(w - 1)
                inj = {}
                if w == 2:
                    inj[4] = lambda: epilogue_compute(0, defer=300)
                if w == 3:
                    inj[0] = lambda: epilogue_compute(1, defer=400)
                    inj[24] = lambda: epilogue_compute(2, defer=150)
                attention_qc(w, inj)
                proj_chunk(w)
            epilogue_read(NQC - 1)
            epilogue_compute(NQC - 1)
            for w in range(NQC):
                epilogue_transpose(w)

        # ============= MLP =================================================
        wfcv = r(wfc[:], "p (m k c) -> p m k c", k=NCP, c=128)
        with tc.tile_pool(name="fc1", bufs=2, space="PSUM") as fps, \
             tc.tile_pool(name="hg", bufs=1) as hgp, \
             tc.tile_pool(name="wfc2", bufs=2) as wf2p, \
             tc.tile_pool(name="y2p", bufs=2) as y2p, \
             tc.tile_pool(name="outp", bufs=1) as outp:
            hgT = hgp.tile([128, NFP, ROWS], BF16, tag="hgT", name="hgT")
            # y2T[p, m2, j, c]: token-major fc2 output blocks
            y2T = outp.tile([128, NCP, 4, 128], BF16, tag="y2T", name="y2T")

            # fc1 in two row-groups: chunks 0-2 (can run while the last
            # chunk's RS is still in flight), then chunk 3.
            def fc1_pass(r0, r1):
                n = r1 - r0
                for m in range(NFP):
                    ps = fps.tile([128, 512], F32, tag="fc")
                    for kt in range(NCP):
                        rhs = (xn2Ta[:, :, kt, :] if r0 == 0
                               else xn2Tb[:, kt, :])
                        nc.tensor.matmul(
                            ps[:, 0:n],
                            wfcv[:, m, kt, :],
                            rhs,
                            start=(kt == 0), stop=(kt == NCP - 1))
                    nc.scalar.activation(hgT[:, m, r0:r1], ps[:, 0:n],
                                         ACTF.Gelu, bias=bfc[:, m:m + 1],
                                         scale=1.0)

            fc1_pass(0, 384)

            osb = [outp.tile([128, C], F32, tag=f"os{j}", name=f"os{j}")
                   for j in range(NQC)]
            y2s = {}

            def fc2_pass(r0, r1, transpose_after):
                n = r1 - r0
                for m2 in range(NCP):
                    wt = wf2p.tile([128, NFP * 128], BF16, tag="wf2")
                    nc.scalar.dma_start(
                        wt[:],
                        w_fc2.ap()[:, m2 * NFP * 128:(m2 + 1) * NFP * 128])
                    ps = fps.tile([128, 512], F32, tag="fc")
                    for k2 in range(NFP):
                        nc.tensor.matmul(
                            ps[:, 0:n], wt[:, k2 * 128:(k2 + 1) * 128],
                            hgT[:, k2, r0:r1],
                            start=(k2 == 0), stop=(k2 == NFP - 1))
                    if r0 == 0:
                        y2 = y2p.tile([128, 512], BF16, tag="y2",
                                      name=f"y2_{m2}")
                        y2s[m2] = y2
                    else:
                        y2 = y2s[m2]
                    nc.vector.tensor_scalar(y2[:, r0:r1], ps[:, 0:n],
                                            bfc2[:, m2:m2 + 1],
                                            None, op0=ALU.add)
                    if transpose_after:
                        nc.sync.dma_start(y2T[:, m2, :, :], y2[:],
                                          transpose=True)
                        for j in range(NQC):
                            nc.vector.tensor_tensor(
                                osb[j][:, m2 * 128:(m2 + 1) * 128],
                                y2T[:, m2, j, :],
                                x2[j][:, m2 * 128:(m2 + 1) * 128],
                                op=ALU.add)

            fc1_pass(384, 512)
            fc2_pass(0, 512, True)

            for j in range(NQC):
                nc.sync.dma_start(out.ap()[j * 128:(j + 1) * 128, :],
                                  osb[j][:])

    nc.compile()
    return nc


_NC_CACHE = None


def _get_program():
    global _NC_CACHE
    if _NC_CACHE is None:
        _NC_CACHE = build_program()
    return _NC_CACHE


def _prepare_in_maps(x, ln1_g, ln1_b, w_attn, b_attn, w_proj, b_proj,
                     ln2_g, ln2_b, w_fc, b_fc, w_fc2, b_fc2):
    bf = ml_dtypes.bfloat16
    x = np.asarray(x, np.float32)
    ln1_g = np.asarray(ln1_g, np.float32); ln1_b = np.asarray(ln1_b, np.float32)
    w_attn = np.asarray(w_attn, np.float32); b_attn = np.asarray(b_attn, np.float32)
    w_proj = np.asarray(w_proj, np.float32); b_proj = np.asarray(b_proj, np.float32)
    ln2_g = np.asarray(ln2_g, np.float32); ln2_b = np.asarray(ln2_b, np.float32)
    w_fc = np.asarray(w_fc, np.float32); b_fc = np.asarray(b_fc, np.float32)
    w_fc2 = np.asarray(w_fc2, np.float32); b_fc2 = np.asarray(b_fc2, np.float32)

    # Fold LayerNorm affine params into the following matmuls (exact).
    w_attn_f = ln1_g[:, None] * w_attn
    b_attn_f = b_attn + ln1_b @ w_attn
    w_fc_f = ln2_g[:, None] * w_fc
    b_fc_f = b_fc + ln2_b @ w_fc

    # staircase mask: m[p, j] = 1 iff j >= p
    jj = np.arange(128)[None, :]
    pp = np.arange(128)[:, None]
    cmask = (jj >= pp).astype(bf)

    # w_fc packed [p][m][kt][c] = w_fc_f[kt*128+p, m*128+c]
    wfc_p = np.ascontiguousarray(
        w_fc_f.reshape(NCP, 128, NFP, 128).transpose(1, 2, 0, 3)
        .reshape(128, -1)).astype(bf)
    # w_fc2 packed [p][m2][k2][c] = w_fc2[k2*128+p, m2*128+c]
    wfc2_p = np.ascontiguousarray(
        w_fc2.reshape(NFP, 128, NCP, 128).transpose(1, 2, 0, 3)
        .reshape(128, -1)).astype(bf)

    shared = {
        "cmask": cmask,
        "w_fc": wfc_p,
        "w_fc2": wfc2_p,
        "b_fc_col": np.ascontiguousarray(b_fc_f.reshape(NFP, 128).T),
        "b_fc2_col": np.ascontiguousarray(b_fc2.reshape(NCP, 128).T),
    }

    in_maps = []
    for c in range(NCORES):
        bidx = c // 4
        g = c % 4
        fsl = slice(g * HGF, (g + 1) * HGF)
        w_q = w_attn_f[:, 0 * C:1 * C][:, fsl]
        w_k = w_attn_f[:, 1 * C:2 * C][:, fsl]
        w_v = w_attn_f[:, 2 * C:3 * C][:, fsl]
        b_q = b_attn_f[0 * C:1 * C][fsl]
        b_k = b_attn_f[1 * C:2 * C][fsl]
        b_v = b_attn_f[2 * C:3 * C][fsl]
        m = dict(shared)
        m["xb"] = np.ascontiguousarray(x[bidx]).astype(bf)
        # own rows: {512*w + 128*g} per chunk window w
        m["xo"] = np.ascontiguousarray(np.concatenate(
            [x[bidx][512 * w + 128 * g:512 * w + 128 * (g + 1)]
             for w in range(NQC)], axis=0) + b_proj[None, :]).astype(bf)
        wqkv = np.concatenate([w_q, w_k, w_v], axis=1)  # [1024, 768]
        m["w_qkv"] = np.ascontiguousarray(
            wqkv.reshape(NCP, 128, 3 * HGF).transpose(1, 0, 2)).astype(bf)
        m["w_pr"] = np.ascontiguousarray(w_proj[fsl, :]).astype(bf)
        m["b_qk_col"] = np.ascontiguousarray(
            np.concatenate([b_q, b_k]).reshape(4, 128).T)
        m["b_v_bc"] = np.ascontiguousarray(
            np.broadcast_to(b_v, (128, HGF))).astype(bf)
        in_maps.append(m)
    return in_maps


def _gather(res):
    y = np.empty((B, T, C), np.float32)
    for c in range(NCORES):
        bidx = c // 4
        g = c % 4
        o = res.results[c]["out"]
        for w in range(NQC):
            y[bidx, 512 * w + 128 * g:512 * w + 128 * (g + 1)] = \
                o[w * 128:(w + 1) * 128]
    return y


def kernel(**inputs):
    in_maps = _prepare_in_maps(**inputs)
    nc = _get_program()
    res = run_bass_kernel_spmd(nc, in_maps, core_ids=list(range(NCORES)))
    return _gather(res)


def run_traced(inputs, **kw):
    """Run with NTFF tracing; returns (output, BassKernelResults)."""
    in_maps = _prepare_in_maps(**inputs)
    nc = _get_program()
    res = run_bass_kernel_spmd(nc, in_maps, core_ids=list(range(NCORES)),
                               trace=True, **kw)
    return _gather(res), res


# revision 63
# speedup vs baseline: 1.0330x; 1.0330x over previous
"""Trainium2 Bass kernel for a dense transformer block (B=2, T=2048, C=1024, H=16).

Sharding (pipelined tensor-parallel attention + chunked ReduceScatter):
  core c -> batch b = c//4, head-group g = c%4 (heads 4g..4g+3).
  After the chunked proj ReduceScatter, core g owns the scattered token
  rows {512*w + 128*g : w in 0..3} of its batch for the MLP phase.

Pipeline (w = 512-token window / query chunk):
  LN1(window w) -> 16x128 xbar DMA transpose -> qkv(window w) ->
  attention for queries w over keys 0..w (block-causal exact,
  live-range trimmed diagonal tiles, single [128,128] staircase mask,
  softmax denominator via a ones-column in v, scores software-pipelined
  one pair ahead of the AV matmuls) -> normalize -> proj partial
  (token-major) -> per-window ReduceScatter(add) over the 4-core batch
  group.  The four chunk collectives run on the collective queue
  underneath the remaining attention compute; only the last chunk's RS
  is exposed.  Residual + LN2 per 128-row chunk (rs reads ride the Pool
  SWDGE queue so their waits never block the SP/ACT queues); fc1 runs
  in two row-groups (chunks 0-2 can start while the last RS is still in
  flight, chunk 3 after); fc2 full width; fc2 outputs are
  xbar-transposed back to token-major and added to the residual.

All transposes are xbar DMA transposes (bf16).  LN rsqrt is ACT Sqrt +
DVE reciprocal up front and a DVE-only Babylonian iteration inside the
attention stream (keeps the exp activation table resident).  LayerNorm
affines and b_proj/b_v are folded into the adjacent matmul weights /
the residual input on the host (exact).  All matmuls are bf16 with
fp32 PSUM accumulation.  PSUM budget in the attention loop:
qkv(2) + scores(4) + av/proj(2) = 8 banks.
"""

import contextlib
from contextlib import ExitStack

import ml_dtypes
import numpy as np

import concourse.bass as bass
import concourse.tile as tile
import concourse.bacc as bacc
import concourse.mybir as mybir
from concourse.bass_utils import run_bass_kernel_spmd

F32 = mybir.dt.float32
BF16 = mybir.dt.bfloat16
ALU = mybir.AluOpType
ACTF = mybir.ActivationFunctionType

B, T, C = 2, 2048, 1024
H, DH = 16, 64
FF = 4096
EPS = 1e-5
NCORES = 8
ROWS = 512            # token rows owned per core (MLP phase)
HG = 4                # heads per core
HGF = HG * DH         # 256 head-group features
NTT = T // 128        # 16 token tiles
NCP = C // 128        # 8 feature partition-tiles of C
NFP = FF // 128       # 32 feature partition-tiles of FF
NQC = T // 512        # 4 query chunks / token windows
VSTRIDE = DH + 1      # v stored with a ones column per head


def r(ap, pat, **kw):
    return ap.rearrange(pat, **kw)


def build_program():
    nc = bacc.Bacc("TRN2", target_bir_lowering=False, debug=False,
                   num_devices=NCORES)

    def din(name, shape, dtype=F32):
        return nc.dram_tensor(name, list(shape), dtype, kind="ExternalInput")

    xb = din("xb", (T, C), BF16)              # LN1 input (bf16 copy of x)
    xo_d = din("xo", (ROWS, C), BF16)         # own rows of x (residual)
    cmask = din("cmask", (128, 128), BF16)    # staircase: m[p,j] = j >= p
    w_qkv = din("w_qkv", (128, NCP, 3 * HGF), BF16)   # [p][kt][q|k|v]
    w_pr = din("w_pr", (HGF, C), BF16)        # proj rows for this head group
    w_fc = din("w_fc", (128, NFP * NCP * 128), BF16)  # [p][m][kt][c]
    w_fc2 = din("w_fc2", (128, NCP * NFP * 128), BF16)  # [p][m2][k2][c]
    b_qk_col = din("b_qk_col", (128, 4))      # q0 q1 k0 k1 bias columns
    b_v_bc = din("b_v_bc", (128, HGF), BF16)
    b_fc_col = din("b_fc_col", (128, NFP))
    b_fc2_col = din("b_fc2_col", (128, NCP))
    out = nc.dram_tensor("out", [ROWS, C], F32, kind="ExternalOutput")

    with tile.TileContext(nc) as tc, ExitStack() as ctx:
        # ---- constants ----
        cpool = ctx.enter_context(tc.tile_pool(name="const", bufs=1))
        bqk = cpool.tile([128, 4], F32, tag="bqk")
        bvbc = cpool.tile([128, HGF], BF16, tag="bvbc")
        bfc = cpool.tile([128, NFP], F32, tag="bfc")
        bfc2 = cpool.tile([128, NCP], F32, tag="bfc2")
        mtile = cpool.tile([128, 128], BF16, tag="mtile")
        epsc = cpool.tile([128, 1], F32, tag="epsc")
        nc.gpsimd.memset(epsc[:], EPS)

        def load_late_consts():
            nc.sync.dma_start(bfc[:], b_fc_col.ap())
            nc.sync.dma_start(bfc2[:], b_fc2_col.ap())

        def rsqrt_eps(spool, var_ap, tag):
            """rc = 1/sqrt(var+eps) via ACT Sqrt + DVE reciprocal."""
            sd = spool.tile([128, 1], F32, tag=f"sd{tag}")
            nc.scalar.activation(sd[:], var_ap, ACTF.Sqrt, bias=epsc[:],
                                 scale=1.0)
            rc = spool.tile([128, 1], F32, tag=f"rc{tag}")
            nc.vector.reciprocal(rc[:], sd[:])
            return rc

        def rsqrt_eps_dve(spool, var_ap, tag):
            """rc = 1/sqrt(var+eps) entirely on DVE (Babylonian iteration;
            var is O(1) here so 3 iterations from s0=(u+1)/2 are exact to
            ~1e-6).  Keeps ACT free of non-exp table sets mid-attention."""
            u = spool.tile([128, 1], F32, tag=f"u{tag}")
            nc.vector.tensor_scalar(u[:], var_ap, EPS, None, op0=ALU.add)
            s = spool.tile([128, 1], F32, tag=f"s{tag}")
            nc.vector.tensor_scalar(s[:], u[:], 1.0, 0.5, op0=ALU.add,
                                    op1=ALU.mult)
            t = spool.tile([128, 1], F32, tag=f"t{tag}")
            m = spool.tile([128, 1], F32, tag=f"m{tag}")
            for _ in range(3):
                nc.vector.reciprocal(t[:], s[:])
                nc.vector.tensor_tensor(m[:], u[:], t[:], op=ALU.mult)
                nc.vector.tensor_scalar(s[:], s[:], m[:], 0.5, op0=ALU.add,
                                        op1=ALU.mult)
            rc = spool.tile([128, 1], F32, tag=f"rc{tag}")
            nc.vector.reciprocal(rc[:], s[:])
            return rc

        def ln_stats(spool, xt, tag):
            """Returns ag [128,2] = (mean, var) over the 1024 free elems."""
            st = spool.tile([128, 12], F32, tag=f"st{tag}")
            nc.vector.bn_stats(st[:, 0:6], xt[:, 0:512])
            nc.vector.bn_stats(st[:, 6:12], xt[:, 512:1024])
            ag = spool.tile([128, 2], F32, tag=f"ag{tag}")
            nc.vector.bn_aggr(ag[:], r(st, "p (c s) -> p c s", s=6))
            return ag

        # persistent SBUF (lives into the MLP phase)
        app = ctx.enter_context(tc.tile_pool(name="attp", bufs=1))
        x2 = [app.tile([128, C], F32, tag=f"x2{i}", name=f"x2{i}")
              for i in range(NQC)]
        # feature-major LN2 output: chunks 0-2 in one tile (fc1 row-group
        # a reads them as one N=384 moving operand), chunk 3 separate
        xn2Ta = app.tile([128, 3, NCP, 128], BF16, tag="xn2Ta", name="xn2Ta")
        xn2Tb = app.tile([128, NCP, 128], BF16, tag="xn2Tb", name="xn2Tb")

        wfcp = ctx.enter_context(tc.tile_pool(name="wfc", bufs=1))
        wfc = wfcp.tile([128, NFP * NCP * 128], BF16, tag="wfc", name="wfc")

        # DRAM bounce buffers for the chunked collective
        drpool = ctx.enter_context(tc.tile_pool(name="dram", bufs=1,
                                                space="DRAM"))
        pp_d = [drpool.tile([ROWS, C], BF16, tag=f"pp{w}", name=f"pp{w}")
                for w in range(NQC)]
        rs_d = [drpool.tile([128, C], BF16, tag=f"rs{w}", name=f"rs{w}")
                for w in range(NQC)]

        with ExitStack() as qctx:
            atb = qctx.enter_context(tc.tile_pool(name="attb", bufs=1))
            kTb = [atb.tile([128, T], BF16, tag=f"kT{i}", name=f"kT{i}")
                   for i in range(2)]
            vb = [atb.tile([128, HG * VSTRIDE], BF16, tag=f"v{i}",
                           name=f"v{i}") for i in range(NTT)]
            qyp = qctx.enter_context(tc.tile_pool(name="qy", bufs=2))
            yyp = qctx.enter_context(tc.tile_pool(name="yy", bufs=1))
            cur = {}

            lnp = qctx.enter_context(tc.tile_pool(name="ln1", bufs=8))
            lnxn = qctx.enter_context(tc.tile_pool(name="ln1x", bufs=6))
            lns = qctx.enter_context(tc.tile_pool(name="ln1s", bufs=8))
            xnTp = qctx.enter_context(tc.tile_pool(name="xnT", bufs=2))
            xnTs = {}
            wqp = qctx.enter_context(tc.tile_pool(name="wq", bufs=1))
            wq = wqp.tile([128, NCP, 3 * HGF], BF16, tag="wq", name="wq")
            wpp = qctx.enter_context(tc.tile_pool(name="wp", bufs=1))
            wp = [wpp.tile([128, C], BF16, tag=f"wp{i}", name=f"wp{i}")
                  for i in range(2)]

            qkps = qctx.enter_context(tc.tile_pool(name="qkps", bufs=2,
                                                   space="PSUM"))
            scps = qctx.enter_context(tc.tile_pool(name="scps", bufs=2,
                                                   space="PSUM"))
            avps = qctx.enter_context(tc.tile_pool(name="avps", bufs=1,
                                                   space="PSUM"))
            atp = qctx.enter_context(tc.tile_pool(name="att", bufs=2))
            smp = qctx.enter_context(tc.tile_pool(name="attsm", bufs=1))
            pep = qctx.enter_context(tc.tile_pool(name="pe", bufs=1))
            esp = qctx.enter_context(tc.tile_pool(name="ep", bufs=3))
            exp2 = qctx.enter_context(tc.tile_pool(name="ep2", bufs=2))
            ess = qctx.enter_context(tc.tile_pool(name="eps", bufs=3))

            xns = {}

            def ln1_compute(w):
                """x loads (SP) + LN1 stats/apply (DVE/ACT) for window w."""
                for tl in range(4):
                    tt = 4 * w + tl
                    xt = lnp.tile([128, C], BF16, tag="x")
                    nc.sync.dma_start(xt[:],
                                      xb.ap()[tt * 128:(tt + 1) * 128, :])
                    if tt == 3:
                        q4 = NCP * 3 * HGF // 4
                        wqf = r(wq[:], "p a b -> p (a b)")
                        wqs = r(w_qkv.ap(), "p a b -> p (a b)")
                        for ci in range(4):
                            nc.sync.dma_start(wqf[:, ci * q4:(ci + 1) * q4],
                                              wqs[:, ci * q4:(ci + 1) * q4])
                        nc.sync.dma_start(bqk[:], b_qk_col.ap())
                        nc.sync.dma_start(bvbc[:], b_v_bc.ap())
                        nc.sync.dma_start(mtile[:], cmask.ap())
                        for i in range(2):
                            nc.sync.dma_start(
                                wp[i][:], w_pr.ap()[i * 128:(i + 1) * 128])
                    ag = ln_stats(lns, xt[:], "a")
                    rc = rsqrt_eps(lns, ag[:, 1:2], "a")
                    nmr = lns.tile([128, 1], F32, tag="nmr")
                    nc.vector.tensor_scalar(nmr[:], ag[:, 0:1], rc[:], -1.0,
                                            op0=ALU.mult, op1=ALU.mult)
                    xn = lnxn.tile([128, C], BF16, tag="xn")
                    nc.scalar.activation(xn[:], xt[:], ACTF.Identity,
                                         bias=nmr[:], scale=rc[:])
                    xns[tt] = xn

            def ln1_transpose(w):
                xnT = xnTp.tile([128, 4, NCP, 128], BF16, tag="xnT",
                                name=f"xnT{w}")
                xnTs[w] = xnT
                for tl in range(4):
                    nc.sync.dma_start(xnT[:, tl, :, :],
                                      xns.pop(4 * w + tl)[:],
                                      transpose=True)

            def qkv_window(w):
                """q,k (feature-major) + v (token-major) for window w."""
                xnT = xnTs.pop(w)
                cur["qT"] = [qyp.tile([128, 512], BF16, tag=f"qTw{i}",
                                      name=f"qT_{w}_{i}") for i in range(2)]
                for m in range(4):
                    ps = qkps.tile([128, 512], F32, tag="qk")
                    for kt in range(NCP):
                        nc.tensor.matmul(
                            ps[:],
                            wq[:, kt, m * 128:(m + 1) * 128],
                            xnT[:, :, kt, :],
                            start=(kt == 0), stop=(kt == NCP - 1))
                    if m < 2:
                        dst = cur["qT"][m][:, :]
                    else:
                        dst = kTb[m % 2][:, w * 512:(w + 1) * 512]
                    nc.vector.tensor_scalar(
                        dst, ps[:],
                        bqk[:, m:m + 1], None, op0=ALU.add)
                for tl in range(4):
                    tt = 4 * w + tl
                    ps = qkps.tile([128, 512], F32, tag="qk")
                    for kt in range(NCP):
                        nc.tensor.matmul(
                            ps[:, 0:HGF],
                            xnT[:, tl, kt, :],
                            wq[:, kt, 2 * HGF:3 * HGF],
                            start=(kt == 0), stop=(kt == NCP - 1))
                    dst = r(vb[tt], "p (h m) -> p h m",
                            m=VSTRIDE)[:, :, 0:DH]
                    nc.vector.tensor_tensor(
                        dst, r(ps[:, 0:HGF], "p (h m) -> p h m", m=DH),
                        r(bvbc[:], "p (h m) -> p h m", m=DH), op=ALU.add)

            def attention_qc(qc, inject={}):
                """Queries [512qc, 512qc+512) for the 4 heads; exact
                block-causal with live-range trimmed diagonal tiles."""
                nkt = 4 * (qc + 1)
                qT = cur["qT"]
                cur["yT"] = [yyp.tile([128, 512], BF16, tag=f"yTw{i}",
                                      name=f"yT_{qc}_{i}") for i in range(2)]

                pairs = [(pt, kp, sub) for pt in range(2)
                         for kp in range(nkt // 2) for sub in range(2)]
                avs = {}
                sc_et = {}

                def segs_of(kp):
                    out = []
                    for j in range(2):
                        kt = 2 * kp + j
                        band = kt - 4 * qc
                        lo = 128 * band if band > 0 else 0
                        out.append((kt, band, lo,
                                    slice(j * 512 + lo, (j + 1) * 512)))
                    return out

                def scores(pair):
                    pt, kp, sub = pair
                    hb = 64 * sub
                    sc = scps.tile([128, 1024], F32, tag="sc")
                    et = atp.tile([128, 1024], BF16, tag="e")
                    sc_et[pair] = (sc, et)
                    for kt, band, lo, seg in segs_of(kp):
                        nc.tensor.matmul(
                            sc[:, seg],
                            kTb[pt][hb:hb + 64, kt * 128:(kt + 1) * 128],
                            qT[pt][hb:hb + 64, lo:512],
                            start=True, stop=True)

                def expmask(pair):
                    pt, kp, sub = pair
                    sc, et = sc_et[pair]
                    sg = segs_of(kp)
                    if sg[0][1] < 0 and sg[1][1] < 0:
                        nc.scalar.activation(et[:, 0:1024], sc[:, 0:1024],
                                             ACTF.Exp, scale=0.125)
                    else:
                        for kt, band, lo, seg in sg:
                            nc.scalar.activation(et[:, seg], sc[:, seg],
                                                 ACTF.Exp, scale=0.125)
                    for kt, band, lo, seg in sg:
                        if band >= 0:
                            j = kt - 2 * kp
                            dsl = slice(j * 512 + lo, j * 512 + lo + 128)
                            nc.vector.tensor_tensor(
                                et[:, dsl], et[:, dsl], mtile[:],
                                op=ALU.mult)

                def av(pair):
                    pt, kp, sub = pair
                    h = 2 * pt + sub
                    sc, et = sc_et.pop(pair)
                    if (pt, sub) not in avs:
                        avs[(pt, sub)] = avps.tile(
                            [128, 512], F32, tag=f"av{sub}",
                            name=f"av_{qc}_{pt}_{sub}")
                    for kt, band, lo, seg in segs_of(kp):
                        nc.tensor.matmul(
                            avs[(pt, sub)][0:VSTRIDE, lo:512],
                            vb[kt][:, h * VSTRIDE:(h + 1) * VSTRIDE],
                            et[:, seg],
                            start=(kt == 0), stop=(kt == nkt - 1),
                            skip_group_check=True)

                def normalize(pt):
                    for sub in range(2):
                        hb = 64 * sub
                        a = avs.pop((pt, sub))
                        rr = smp.tile([1, 512], F32, tag="rr")
                        nc.vector.reciprocal(rr[:], a[DH:DH + 1, :])
                        bc = smp.tile([64, 512], F32, tag="bc")
                        nc.gpsimd.partition_broadcast(bc[:], rr[:])
                        nc.vector.tensor_tensor(
                            cur["yT"][pt][hb:hb + 64, :],
                            a[0:DH, :], bc[:], op=ALU.mult)

                scores(pairs[0])
                for i, pair in enumerate(pairs):
                    fn = inject.pop(i, None)
                    if fn is not None:
                        fn()
                    expmask(pair)
                    if i + 1 < len(pairs):
                        scores(pairs[i + 1])
                    av(pair)
                    if pair[0] == 0 and i + 1 < len(pairs) \
                            and pairs[i + 1][0] == 1:
                        normalize(0)
                normalize(1)

            def proj_chunk(w):
                """Token-major proj partial for window w + chunk RS."""
                pe = pep.tile([128, 4, C], BF16, tag="pe")
                yT = cur["yT"]
                for tl in range(4):
                    for cc in range(2):
                        ps = avps.tile([128, 512], F32,
                                       tag=f"av{(2 * tl + cc) % 2}",
                                       name=f"pj_{w}_{tl}_{cc}")
                        for i in range(2):
                            nc.tensor.matmul(
                                ps[:],
                                yT[i][:, tl * 128:(tl + 1) * 128],
                                wp[i][:, cc * 512:(cc + 1) * 512],
                                start=(i == 0), stop=(i == 1))
                        nc.vector.tensor_copy(
                            pe[:, tl, cc * 512:(cc + 1) * 512], ps[:])

                # pp writes from the Pool SWDGE queue (waits never block the
                # SP/ACT streams), in halves so the last write -- which
                # gates the collective trigger -- is only 512 KB
                for h2 in range(2):
                    nc.gpsimd.dma_start(
                        r(pp_d[w][h2 * 256:(h2 + 1) * 256, :],
                          "(o p) c -> p o c", p=128),
                        pe[:, 2 * h2:2 * h2 + 2, :])
                nc.gpsimd.collective_compute(
                    "ReduceScatter", ALU.add,
                    replica_groups=[[0, 1, 2, 3], [4, 5, 6, 7]],
                    ins=[pp_d[w].opt()], outs=[rs_d[w].opt()])

            xn2s = {}

            rs_sbs = {}

            def epilogue_read(w):
                """rs read (Pool swdge: reached only after the next chunk's
                collective trigger, so its wait never blocks) + xo load."""
                rs_sb = esp.tile([128, C], BF16, tag="rs")
                nc.gpsimd.dma_start(rs_sb[:], rs_d[w][:, :])
                xot = esp.tile([128, C], BF16, tag="xot")
                nc.sync.dma_start(xot[:],
                                  xo_d.ap()[w * 128:(w + 1) * 128, :])
                rs_sbs[w] = (rs_sb, xot)

            def epilogue_compute(w, defer=0):
                """residual + LN2 (DVE only).  ``defer`` pushes the ops'
                scheduler priority later so their RS-gated waits never clog
                the in-order DVE queue ahead of attention work."""
                with tc.high_priority(offset=-defer) if defer else \
                        contextlib.nullcontext():
                    _epilogue_compute(w)

            def _epilogue_compute(w):
                rs_sb, xot = rs_sbs.pop(w)
                nc.vector.tensor_tensor(x2[w][:], rs_sb[:], xot[:],
                                        op=ALU.add)
                ag = ln_stats(ess, x2[w][:], "e")
                rc = rsqrt_eps_dve(ess, ag[:, 1:2], "e")
                xn2 = exp2.tile([128, C], BF16, tag="xn2")
                nc.vector.tensor_scalar(xn2[:], x2[w][:], ag[:, 0:1], rc[:],
                                        op0=ALU.subtract, op1=ALU.mult)
                xn2s[w] = xn2

            def epilogue_transpose(w):
                dst = xn2Ta[:, w, :, :] if w < 3 else xn2Tb[:, :, :]
                nc.sync.dma_start(dst, xn2s.pop(w)[:], transpose=True)

            # ---------------- pipelined main loop ----------------
            for tt in range(NTT):
                nc.gpsimd.memset(
                    r(vb[tt], "p (h m) -> p h m",
                      m=VSTRIDE)[:, :, DH:DH + 1], 1.0)
            ln1_compute(0)
            ln1_compute(1)
            for w in range(NQC):
                if 1 <= w < NQC - 1:
                    ln1_compute(w + 1)
                ln1_transpose(w)
                qkv_window(w)
                if w == NQC - 1:
                    nch = 8
                    csz = NFP * NCP * 128 // nch
                    for ci in range(nch):
                        nc.sync.dma_start(wfc[:, ci * csz:(ci + 1) * csz],
                                          w_fc.ap()[:, ci * csz:(ci + 1) * csz])
                    load_late_consts()
                if w >= 1:
                    epilogue_read(w - 1)
                inj = {}
                if w == 2:
                    inj[4] = lambda: epilogue_compute(0, defer=300)
                if w == 3:
                    inj[0] = lambda: epilogue_compute(1, defer=400)
                    inj[24] = lambda: epilogue_compute(2, defer=150)
                attention_qc(w, inj)
                proj_chunk(w)
            epilogue_read(NQC - 1)
            epilogue_compute(NQC - 1)
            for w in range(NQC):
                epilogue_transpose(w)

        # ============= MLP =================================================
        wfcv = r(wfc[:], "p (m k c) -> p m k c", k=NCP, c=128)
        with tc.tile_pool(name="fc1", bufs=6, space="PSUM") as fps, \
             tc.tile_pool(name="hg", bufs=1) as hgp, \
             tc.tile_pool(name="wfc2", bufs=2) as wf2p, \
             tc.tile_pool(name="y2p", bufs=2) as y2p, \
             tc.tile_pool(name="outp", bufs=1) as outp:
            hgT = hgp.tile([128, NFP, ROWS], BF16, tag="hgT", name="hgT")
            # y2T[p, m2, j, c]: token-major fc2 output blocks
            y2T = outp.tile([128, NCP, 4, 128], BF16, tag="y2T", name="y2T")

            # fc1 in two row-groups: chunks 0-2 (can run while the last
            # chunk's RS is still in flight), then chunk 3.
            def fc1_pass(r0, r1):
                n = r1 - r0
                for m in range(NFP):
                    ps = fps.tile([128, 512], F32, tag="fc")
                    for kt in range(NCP):
                        rhs = (xn2Ta[:, :, kt, :] if r0 == 0
                               else xn2Tb[:, kt, :])
                        nc.tensor.matmul(
                            ps[:, 0:n],
                            wfcv[:, m, kt, :],
                            rhs,
                            start=(kt == 0), stop=(kt == NCP - 1))
                    nc.scalar.activation(hgT[:, m, r0:r1], ps[:, 0:n],
                                         ACTF.Gelu, bias=bfc[:, m:m + 1],
                                         scale=1.0)

            fc1_pass(0, 384)

            osb = [outp.tile([128, C], F32, tag=f"os{j}", name=f"os{j}")
                   for j in range(NQC)]
            y2s = {}

            def fc2_pass(r0, r1, transpose_after):
                n = r1 - r0
                for m2 in range(NCP):
                    wt = wf2p.tile([128, NFP * 128], BF16, tag="wf2")
                    nc.scalar.dma_start(
                        wt[:],
                        w_fc2.ap()[:, m2 * NFP * 128:(m2 + 1) * NFP * 128])
                    ps = fps.tile([128, 512], F32, tag="fc")
                    for k2 in range(NFP):
                        nc.tensor.matmul(
                            ps[:, 0:n], wt[:, k2 * 128:(k2 + 1) * 128],
                            hgT[:, k2, r0:r1],
                            start=(k2 == 0), stop=(k2 == NFP - 1))
                    if r0 == 0:
                        y2 = y2p.tile([128, 512], BF16, tag="y2",
                                      name=f"y2_{m2}")
                        y2s[m2] = y2
                    else:
                        y2 = y2s[m2]
                    nc.vector.tensor_scalar(y2[:, r0:r1], ps[:, 0:n],
                                            bfc2[:, m2:m2 + 1],
                                            None, op0=ALU.add)
                    if transpose_after:
                        nc.sync.dma_start(y2T[:, m2, :, :], y2[:],
                                          transpose=True)
                        for j in range(NQC):
                            nc.vector.tensor_tensor(
                                osb[j][:, m2 * 128:(m2 + 1) * 128],
                                y2T[:, m2, j, :],
                                x2[j][:, m2 * 128:(m2 + 1) * 128],
                                op=ALU.add)

            fc1_pass(384, 512)
            fc2_pass(0, 512, True)

            for j in range(NQC):
                nc.sync.dma_start(out.ap()[j * 128:(j + 1) * 128, :],
                                  osb[j][:])

    nc.compile()
    return nc


_NC_CACHE = None


def _get_program():
    global _NC_CACHE
    if _NC_CACHE is None:
        _NC_CACHE = build_program()
    return _NC_CACHE


def _prepare_in_maps(x, ln1_g, ln1_b, w_attn, b_attn, w_proj, b_proj,
                     ln2_g, ln2_b, w_fc, b_fc, w_fc2, b_fc2):
    bf = ml_dtypes.bfloat16
    x = np.asarray(x, np.float32)
    ln1_g = np.asarray(ln1_g, np.float32); ln1_b = np.asarray(ln1_b, np.float32)
    w_attn = np.asarray(w_attn, np.float32); b_attn = np.asarray(b_attn, np.float32)
    w_proj = np.asarray(w_proj, np.float32); b_proj = np.asarray(b_proj, np.float32)
    ln2_g = np.asarray(ln2_g, np.float32); ln2_b = np.asarray(ln2_b, np.float32)
    w_fc = np.asarray(w_fc, np.float32); b_fc = np.asarray(b_fc, np.float32)
    w_fc2 = np.asarray(w_fc2, np.float32); b_fc2 = np.asarray(b_fc2, np.float32)

    # Fold LayerNorm affine params into the following matmuls (exact).
    w_attn_f = ln1_g[:, None] * w_attn
    b_attn_f = b_attn + ln1_b @ w_attn
    w_fc_f = ln2_g[:, None] * w_fc
    b_fc_f = b_fc + ln2_b @ w_fc

    # staircase mask: m[p, j] = 1 iff j >= p
    jj = np.arange(128)[None, :]
    pp = np.arange(128)[:, None]
    cmask = (jj >= pp).astype(bf)

    # w_fc packed [p][m][kt][c] = w_fc_f[kt*128+p, m*128+c]
    wfc_p = np.ascontiguousarray(
        w_fc_f.reshape(NCP, 128, NFP, 128).transpose(1, 2, 0, 3)
        .reshape(128, -1)).astype(bf)
    # w_fc2 packed [p][m2][k2][c] = w_fc2[k2*128+p, m2*128+c]
    wfc2_p = np.ascontiguousarray(
        w_fc2.reshape(NFP, 128, NCP, 128).transpose(1, 2, 0, 3)
        .reshape(128, -1)).astype(bf)

    shared = {
        "cmask": cmask,
        "w_fc": wfc_p,
        "w_fc2": wfc2_p,
        "b_fc_col": np.ascontiguousarray(b_fc_f.reshape(NFP, 128).T),
        "b_fc2_col": np.ascontiguousarray(b_fc2.reshape(NCP, 128).T),
    }

    in_maps = []
    for c in range(NCORES):
        bidx = c // 4
        g = c % 4
        fsl = slice(g * HGF, (g + 1) * HGF)
        w_q = w_attn_f[:, 0 * C:1 * C][:, fsl]
        w_k = w_attn_f[:, 1 * C:2 * C][:, fsl]
        w_v = w_attn_f[:, 2 * C:3 * C][:, fsl]
        b_q = b_attn_f[0 * C:1 * C][fsl]
        b_k = b_attn_f[1 * C:2 * C][fsl]
        b_v = b_attn_f[2 * C:3 * C][fsl]
        m = dict(shared)
        m["xb"] = np.ascontiguousarray(x[bidx]).astype(bf)
        # own rows: {512*w + 128*g} per chunk window w
        m["xo"] = np.ascontiguousarray(np.concatenate(
            [x[bidx][512 * w + 128 * g:512 * w + 128 * (g + 1)]
             for w in range(NQC)], axis=0) + b_proj[None, :]).astype(bf)
        wqkv = np.concatenate([w_q, w_k, w_v], axis=1)  # [1024, 768]
        m["w_qkv"] = np.ascontiguousarray(
            wqkv.reshape(NCP, 128, 3 * HGF).transpose(1, 0, 2)).astype(bf)
        m["w_pr"] = np.ascontiguousarray(w_proj[fsl, :]).astype(bf)
        m["b_qk_col"] = np.ascontiguousarray(
            np.concatenate([b_q, b_k]).reshape(4, 128).T)
        m["b_v_bc"] = np.ascontiguousarray(
            np.broadcast_to(b_v, (128, HGF))).astype(bf)
        in_maps.append(m)
    return in_maps


def _gather(res):
    y = np.empty((B, T, C), np.float32)
    for c in range(NCORES):
        bidx = c // 4
        g = c % 4
        o = res.results[c]["out"]
        for w in range(NQC):
            y[bidx, 512 * w + 128 * g:512 * w + 128 * (g + 1)] = \
                o[w * 128:(w + 1) * 128]
    return y


def kernel(**inputs):
    in_maps = _prepare_in_maps(**inputs)
    nc = _get_program()
    res = run_bass_kernel_spmd(nc, in_maps, core_ids=list(range(NCORES)))
    return _gather(res)


def run_traced(inputs, **kw):
    """Run with NTFF tracing; returns (output, BassKernelResults)."""
    in_maps = _prepare_in_maps(**inputs)
    nc = _get_program()
    res = run_bass_kernel_spmd(nc, in_maps, core_ids=list(range(NCORES)),
                               trace=True, **kw)
    return _gather(res), res
